# revision 1
# baseline (speedup 1.0000x reference)
"""2-layer GCN (GCNConv -> ReLU -> GCNConv -> log_softmax) on 8 TRN2 NeuronCores.

Strategy (dest-sharded message passing):
  - Nodes are degree-sorted and assigned round-robin to the 8 cores, so every
    core owns 12500 destinations with a matching degree profile and ~1/8 of
    the edges. Edges are partitioned by destination core.
  - GCN normalization is factored: with dinv = deg^-1/2,
        out[r] = dinv[r] * sum_{e: row_e=r} dinv[col_e] * h[col_e]
    so each layer propagates the pre-scaled features h~ = dinv * h and
    rescales the segment sums by dinv on the destination side.
  - Layer algebra is reassociated so both propagations run at width 64:
        h1 = relu(P(x@W1) + b1),   out = (P h1) @ W2 + b2,
    where P is the shared normalized-adjacency operator.
  - Per layer: each core computes its shard of the propagation source,
    AllGathers it into a full [100352, 64] HBM copy, then uses indirect
    (gather) DMA with a destination-major padded index table to pull source
    rows into SBUF as [128 dests, slots, 64]; a strided vector-engine reduce
    collapses the slot axis, giving the segment sums per destination tile.
  - Index tables are built on the host from edge_index (index metadata only;
    all feature math runs on device) and are identical for both layers.
"""

import numpy as np

import concourse.bass as bass
import concourse.bacc as bacc
import concourse.mybir as mybir
import concourse.tile as tile
from concourse.bass_utils import run_bass_kernel_spmd
from concourse.masks import make_identity

N_CORES = 8
P = 128
N_NODES = 100000
F0, F1, F2 = 128, 64, 32
T = (N_NODES // N_CORES + P - 1) // P  # 98 destination tiles per core
SH = T * P                             # 12544 padded shard rows
NFULL = N_CORES * SH                   # 100352 gather-source rows
GS_CAP = 128                           # max gather slots per group
XCHUNK_TILES = 14                      # xT streaming chunk (14 tiles = 1792 cols)

_CACHE = {}


def _preprocess(x, edge_index, W1, b1, W2, b2):
    ei = np.asarray(edge_index)
    n = N_NODES
    loops = np.arange(n, dtype=np.int64)
    row = np.concatenate([ei[0].astype(np.int64), loops])
    col = np.concatenate([ei[1].astype(np.int64), loops])

    deg = np.bincount(row, minlength=n)
    dinv = np.zeros(n, np.float32)
    nz = deg > 0
    dinv[nz] = 1.0 / np.sqrt(deg[nz].astype(np.float64)).astype(np.float32)

    # degree-sorted round-robin node -> (core, pos)
    order = np.argsort(-deg, kind="stable")
    core_of = np.empty(n, np.int32)
    pos_of = np.empty(n, np.int32)
    core_of[order] = (np.arange(n) % N_CORES).astype(np.int32)
    pos_of[order] = (np.arange(n) // N_CORES).astype(np.int32)
    src_row = core_of.astype(np.int64) * SH + pos_of  # gather-source row per node
    pad_row = SH - 1  # core 0's last padded row: always zero
    assert (n + N_CORES - 1) // N_CORES <= SH - 1

    # per-edge destination placement
    e_core = core_of[row]
    e_pos = pos_of[row]
    e_tile = e_pos // P
    e_part = e_pos % P
    cnt = np.zeros((N_CORES, T, P), np.int32)
    np.add.at(cnt, (e_core, e_tile, e_part), 1)
    S_t = cnt.max(axis=(0, 2)).astype(np.int64)  # shared slot count per tile
    S_t = np.maximum(S_t, 1)
    off_t = np.zeros(T + 1, np.int64)
    off_t[1:] = np.cumsum(S_t)
    s_total = int(off_t[-1])

    idx = np.full((N_CORES, P, s_total), pad_row, np.int32)
    ek = (e_core.astype(np.int64) * T + e_tile) * P + e_part
    es = np.argsort(ek, kind="stable")
    ek_sorted = ek[es]
    starts = np.r_[0, np.flatnonzero(np.diff(ek_sorted)) + 1]
    group_ids = np.repeat(
        np.arange(len(starts)), np.diff(np.r_[starts, len(ek_sorted)])
    )
    slot = np.arange(len(ek_sorted)) - starts[group_ids]
    idx[e_core[es], e_part[es], off_t[e_tile[es]] + slot] = src_row[col[es]]

    # node data in (core, pos) layout
    perm = np.argsort(core_of.astype(np.int64) * SH + pos_of, kind="stable")
    node_at = np.full((N_CORES, SH), -1, np.int64)
    node_at[core_of[perm], pos_of[perm]] = perm
    valid = node_at >= 0

    x = np.asarray(x, np.float32)
    xs = np.zeros((N_CORES, SH, F0), np.float32)
    xs[valid] = x[node_at[valid]]
    dinvs = np.zeros((N_CORES, SH), np.float32)
    dinvs[valid] = dinv[node_at[valid]]

    in_maps = []
    b1r = np.broadcast_to(np.asarray(b1, np.float32), (P, F1)).copy()
    b2r = np.broadcast_to(np.asarray(b2, np.float32), (P, F2)).copy()
    w1 = np.asarray(W1, np.float32)
    w2 = np.asarray(W2, np.float32)
    for c in range(N_CORES):
        in_maps.append(
            {
                "xT": np.ascontiguousarray(xs[c].T),                 # [F0, SH]
                "w1": w1,                                            # [F0, F1]
                "w2": w2,                                            # [F1, F2]
                "b1r": b1r,                                          # [P, F1]
                "b2r": b2r,                                          # [P, F2]
                "dinvT": np.ascontiguousarray(
                    dinvs[c].reshape(T, P).T                         # [P, T]
                ),
                "idx": np.ascontiguousarray(idx[c]),                 # [P, s_total]
            }
        )

    # gather groups: contiguous tile runs with slot sum <= GS_CAP
    groups = []
    t0 = 0
    while t0 < T:
        t1 = t0 + 1
        while t1 < T and off_t[t1 + 1] - off_t[t0] <= GS_CAP:
            t1 += 1
        groups.append((t0, t1, int(off_t[t0])))
        t0 = t1
    meta = {
        "S_t": tuple(int(s) for s in S_t),
        "off_t": off_t,
        "groups": groups,
        "s_total": s_total,
    }
    return in_maps, meta, node_at, valid


def _build(meta):
    S_t = meta["S_t"]
    off_t = meta["off_t"]
    groups = meta["groups"]
    s_total = meta["s_total"]
    gs_max = max(int(off_t[t1] - off_t[t0]) for t0, t1, _ in groups)

    nc = bacc.Bacc("TRN2", target_bir_lowering=False, debug=False,
                   num_devices=N_CORES)
    f32 = mybir.dt.float32
    xT_d = nc.dram_tensor("xT", [F0, SH], f32, kind="ExternalInput")
    w1_d = nc.dram_tensor("w1", [F0, F1], f32, kind="ExternalInput")
    w2_d = nc.dram_tensor("w2", [F1, F2], f32, kind="ExternalInput")
    b1_d = nc.dram_tensor("b1r", [P, F1], f32, kind="ExternalInput")
    b2_d = nc.dram_tensor("b2r", [P, F2], f32, kind="ExternalInput")
    dinv_d = nc.dram_tensor("dinvT", [P, T], f32, kind="ExternalInput")
    idx_d = nc.dram_tensor("idx", [P, s_total], mybir.dt.int32,
                           kind="ExternalInput")
    out_d = nc.dram_tensor("out", [SH, F2], f32, kind="ExternalOutput")

    rg = [list(range(N_CORES))]

    with tile.TileContext(nc) as tc:
        with (
            tc.tile_pool(name="const", bufs=1) as cpool,
            tc.tile_pool(name="acc", bufs=1) as apool,
            tc.tile_pool(name="xs", bufs=2) as xpool,
            tc.tile_pool(name="gb", bufs=2) as gpool,
            tc.tile_pool(name="tmp", bufs=4) as tpool,
            tc.tile_pool(name="sm", bufs=4) as smpool,
            tc.tile_pool(name="ps", bufs=2, space="PSUM") as ppool,
            tc.tile_pool(name="dram", bufs=1, space="DRAM") as dpool,
        ):
            # constants
            w1_s = cpool.tile([F0, F1], f32)
            nc.sync.dma_start(w1_s[:], w1_d[:])
            w2_s = cpool.tile([F1, F2], f32)
            nc.sync.dma_start(w2_s[:], w2_d[:])
            b1_s = cpool.tile([P, F1], f32)
            nc.sync.dma_start(b1_s[:], b1_d[:])
            b2_s = cpool.tile([P, F2], f32)
            nc.sync.dma_start(b2_s[:], b2_d[:])
            dinv_s = cpool.tile([P, T], f32)
            nc.sync.dma_start(dinv_s[:], dinv_d[:])
            idx_s = cpool.tile([P, s_total], mybir.dt.int32)
            nc.sync.dma_start(idx_s[:], idx_d[:])
            ident = cpool.tile([P, P], f32)
            make_identity(nc, ident[:])

            bounce1 = dpool.tile([SH, F1], f32)
            bounce2 = dpool.tile([SH, F1], f32)
            y_full = dpool.tile([NFULL, F1], f32)
            h_full = dpool.tile([NFULL, F1], f32)

            # ---- layer 1 shard matmul: ysh = dinv * (x @ W1) ----
            ysh = apool.tile([P, T * F1], f32)
            n_chunks = (T + XCHUNK_TILES - 1) // XCHUNK_TILES
            for ci in range(n_chunks):
                ct0 = ci * XCHUNK_TILES
                ct1 = min(ct0 + XCHUNK_TILES, T)
                xc = xpool.tile([F0, XCHUNK_TILES * P], f32, tag="xc")
                nc.sync.dma_start(
                    xc[:, : (ct1 - ct0) * P], xT_d[:, ct0 * P : ct1 * P]
                )
                for t in range(ct0, ct1):
                    zp = ppool.tile([P, F1], f32, tag="zp", space="PSUM")
                    nc.tensor.matmul(
                        out=zp[:],
                        lhsT=xc[:, (t - ct0) * P : (t - ct0 + 1) * P],
                        rhs=w1_s[:],
                        start=True,
                        stop=True,
                    )
                    nc.vector.tensor_scalar_mul(
                        ysh[:, t * F1 : (t + 1) * F1], zp[:],
                        dinv_s[:, t : t + 1],
                    )
            nc.gpsimd.dma_start(
                bounce1[:].rearrange("(t p) f -> p t f", p=P),
                ysh[:].rearrange("p (t f) -> p t f", f=F1),
            )
            nc.gpsimd.collective_compute(
                "AllGather",
                mybir.AluOpType.bypass,
                ins=[bounce1.opt()],
                outs=[y_full.opt()],
                replica_groups=rg,
            )

            # ---- layer 1 propagate: h1t = relu(dinv*(dinv*seg + b1)) ----
            h1t = apool.tile([P, T * F1], f32)
            for t0, t1, off in groups:
                gsg = int(off_t[t1] - off_t[t0])
                gb = gpool.tile([P, gs_max * F1], f32, tag="gb")
                nc.gpsimd.indirect_dma_start(
                    out=gb[:, : gsg * F1],
                    out_offset=None,
                    in_=y_full[:],
                    in_offset=bass.IndirectOffsetOnAxis(
                        ap=idx_s[:, off : off + gsg], axis=0
                    ),
                )
                for t in range(t0, t1):
                    st = S_t[t]
                    lo = int(off_t[t]) - off
                    seg = tpool.tile([P, F1], f32, tag="seg")
                    nc.vector.tensor_reduce(
                        out=seg[:],
                        in_=gb[:, lo * F1 : (lo + st) * F1].rearrange(
                            "p (s f) -> p f s", f=F1
                        ),
                        axis=mybir.AxisListType.X,
                        op=mybir.AluOpType.add,
                    )
                    y1 = tpool.tile([P, F1], f32, tag="y1")
                    nc.vector.tensor_scalar_mul(
                        y1[:], seg[:], dinv_s[:, t : t + 1]
                    )
                    nc.vector.tensor_tensor(
                        out=y1[:], in0=y1[:], in1=b1_s[:],
                        op=mybir.AluOpType.add,
                    )
                    nc.scalar.activation(
                        out=h1t[:, t * F1 : (t + 1) * F1],
                        in_=y1[:],
                        func=mybir.ActivationFunctionType.Relu,
                        scale=dinv_s[:, t : t + 1],
                    )
            nc.gpsimd.dma_start(
                bounce2[:].rearrange("(t p) f -> p t f", p=P),
                h1t[:].rearrange("p (t f) -> p t f", f=F1),
            )
            nc.gpsimd.collective_compute(
                "AllGather",
                mybir.AluOpType.bypass,
                ins=[bounce2.opt()],
                outs=[h_full.opt()],
                replica_groups=rg,
            )

            # ---- layer 2 propagate + matmul + log_softmax ----
            outacc = apool.tile([P, T * F2], f32)
            for t0, t1, off in groups:
                gsg = int(off_t[t1] - off_t[t0])
                gb = gpool.tile([P, gs_max * F1], f32, tag="gb")
                nc.gpsimd.indirect_dma_start(
                    out=gb[:, : gsg * F1],
                    out_offset=None,
                    in_=h_full[:],
                    in_offset=bass.IndirectOffsetOnAxis(
                        ap=idx_s[:, off : off + gsg], axis=0
                    ),
                )
                for t in range(t0, t1):
                    st = S_t[t]
                    lo = int(off_t[t]) - off
                    seg = tpool.tile([P, F1], f32, tag="seg")
                    nc.vector.tensor_reduce(
                        out=seg[:],
                        in_=gb[:, lo * F1 : (lo + st) * F1].rearrange(
                            "p (s f) -> p f s", f=F1
                        ),
                        axis=mybir.AxisListType.X,
                        op=mybir.AluOpType.add,
                    )
                    zpre = tpool.tile([P, F1], f32, tag="zpre")
                    nc.vector.tensor_scalar_mul(
                        zpre[:], seg[:], dinv_s[:, t : t + 1]
                    )
                    tp = ppool.tile([F1, P], f32, tag="tp", space="PSUM")
                    nc.tensor.transpose(out=tp[:], in_=zpre[:], identity=ident[:])
                    zpreT = tpool.tile([F1, P], f32, tag="zpreT")
                    nc.vector.tensor_copy(zpreT[:], tp[:])
                    z2p = ppool.tile([P, F2], f32, tag="z2p", space="PSUM")
                    nc.tensor.matmul(
                        out=z2p[:], lhsT=zpreT[:], rhs=w2_s[:],
                        start=True, stop=True,
                    )
                    z2 = smpool.tile([P, F2], f32, tag="z2")
                    nc.vector.tensor_tensor(
                        out=z2[:], in0=z2p[:], in1=b2_s[:],
                        op=mybir.AluOpType.add,
                    )
                    negm = smpool.tile([P, 1], f32, tag="negm")
                    nc.vector.tensor_reduce(
                        out=negm[:], in_=z2[:],
                        axis=mybir.AxisListType.X,
                        op=mybir.AluOpType.max, negate=True,
                    )
                    ex = smpool.tile([P, F2], f32, tag="ex")
                    sume = smpool.tile([P, 1], f32, tag="sume")
                    nc.scalar.activation(
                        out=ex[:], in_=z2[:],
                        func=mybir.ActivationFunctionType.Exp,
                        bias=negm[:, 0:1], accum_out=sume[:],
                    )
                    lse = smpool.tile([P, 1], f32, tag="lse")
                    nc.scalar.activation(
                        out=lse[:], in_=sume[:],
                        func=mybir.ActivationFunctionType.Ln,
                    )
                    nc.vector.tensor_scalar(
                        out=outacc[:, t * F2 : (t + 1) * F2],
                        in0=z2[:],
                        scalar1=negm[:, 0:1],
                        scalar2=lse[:, 0:1],
                        op0=mybir.AluOpType.add,
                        op1=mybir.AluOpType.subtract,
                    )
            nc.gpsimd.dma_start(
                out_d[:].rearrange("(t p) f -> p t f", p=P),
                outacc[:].rearrange("p (t f) -> p t f", f=F2),
            )
    nc.compile()
    return nc


def kernel(x, edge_index, W1, b1, W2, b2):
    in_maps, meta, node_at, valid = _preprocess(x, edge_index, W1, b1, W2, b2)
    key = (meta["S_t"], tuple(meta["groups"]))
    if key not in _CACHE:
        _CACHE[key] = _build(meta)
    nc = _CACHE[key]
    res = run_bass_kernel_spmd(nc, in_maps, core_ids=list(range(N_CORES)))
    out = np.zeros((N_NODES, F2), np.float32)
    for c in range(N_CORES):
        oc = res.results[c]["out"]
        out[node_at[c][valid[c]]] = oc[valid[c]]
    return out
